# revision 1
# baseline (speedup 1.0000x reference)
"""Trainium2 Bass kernel: 3D 'same' convolution (implicit GEMM).

Problem: x (4, 64, 24, 24, 24) f32, weight (1, 128, 1728) f32
         -> out (4, 128, 24, 24, 24) f32  (SAME conv3d, k=3)

Sharding (8 cores): batch (4) x z-halves (2). Each core computes
out[b, :, z0:z0+12] for its (b, zh) shard; no inter-core communication.

Per-core algorithm: 27-tap implicit GEMM. The PE array is row-tiled
64x128: partitions 0-63 (tile_position (0,0)) and 64-127 ((64,0)) hold
identical copies of the zero-padded input window and process disjoint
halves of the 27 taps concurrently, accumulating into two separate PSUM
banks which are summed at evacuation (ACT copy + DVE add). Matmuls run
in float32r (FP22 multiply, fp32 PSUM accumulate) at ~1 column/cycle.

The padded input window (14 z-planes) is loaded as two overlapping
8-plane chunks so the second chunk's DMA hides under the first chunk's
matmuls. Output tiles are one z-plane x 21 y-rows x 24 (N=504, 2D
access pattern); the y=21..23 remainder rows are batched across 6
z-planes (N=432) per chunk.
"""

import sys

if "/opt/trn_rl_repo" not in sys.path:
    sys.path.insert(0, "/opt/trn_rl_repo")

import numpy as np

CIN, COUT, K = 64, 128, 3
DHW = 24  # cubic spatial extent
ZS = 12  # z-planes per shard
NP = 14  # padded z-planes per shard window (ZS + 2 halo)
PW = 26  # padded y/x extent
N_CORES = 8

# tap order: all 27 (dz, dy, dx)
TAPS = [(dz, dy, dx) for dz in range(3) for dy in range(3) for dx in range(3)]
N_T0 = 14  # taps on PE row-tile (0,0); the rest go to (64,0)


def _build_program(loop_n=None):
    """Build the SPMD Bass program (one NeuronCore's view).

    loop_n: if set, wrap the whole body in a hardware For_i loop with
    that many iterations (used by test.py for wall-clock timing).
    """
    import concourse.tile as tile
    from concourse import bacc, mybir

    F32 = mybir.dt.float32
    F32R = mybir.dt.float32r

    t0_taps = TAPS[:N_T0]
    t8_taps = TAPS[N_T0:]

    nc = bacc.Bacc("TRN2")
    x_in = nc.declare_dram_parameter("x", [128, NP, PW, PW], F32R, isOutput=False)
    wk_in = nc.declare_dram_parameter("wk", [128, N_T0, 128], F32R, isOutput=False)
    y_out = nc.declare_dram_parameter("y", [128, ZS, DHW, DHW], F32, isOutput=True)

    with tile.TileContext(nc) as tc:
        with (
            tc.tile_pool(name="xw", bufs=1) as xw_pool,
            tc.tile_pool(name="ps", bufs=3, space="PSUM") as ps_pool,
            tc.tile_pool(name="ob", bufs=3) as ob_pool,
        ):

            def body(_iv=None):
                W = xw_pool.tile([128, N_T0, 128], F32R, name="W")
                nc.sync.dma_start(out=W[:], in_=wk_in[:])
                XA = xw_pool.tile([128, 8, PW, PW], F32R, name="XA")
                XB = xw_pool.tile([128, 8, PW, PW], F32R, name="XB")
                nc.sync.dma_start(out=XA[:], in_=x_in[:, 0:8])
                nc.sync.dma_start(out=XB[:], in_=x_in[:, 6:14])

                # output tiles: ("plane", chunk, zoff, z) N=504 (21x24, 2D AP)
                #           or ("rem", chunk, zoff, None) N=432 (6x3x24, 3D)
                tiles = (
                    [("plane", XA, 0, z) for z in range(6)]
                    + [("rem", XA, 0, None)]
                    + [("plane", XB, 6, z) for z in range(6, 12)]
                    + [("rem", XB, 6, None)]
                )

                def rhs_ap(X, zoff, kind, z, dz, dy, dx, lo, hi):
                    if kind == "plane":
                        return X[lo:hi, z - zoff + dz, dy : dy + 21, dx : dx + 24]
                    return X[lo:hi, dz : dz + 6, 21 + dy : 24 + dy, dx : dx + 24]

                for kind, X, zoff, z in tiles:
                    n = 504 if kind == "plane" else 432
                    ps0 = ps_pool.tile([128, 512], F32, name="ps0", tag="ps0")
                    ps1 = ps_pool.tile([128, 512], F32, name="ps1", tag="ps1")
                    n0, n1 = len(t0_taps), len(t8_taps)
                    for i in range(n0):
                        dz, dy, dx = t0_taps[i]
                        nc.tensor.matmul(
                            ps0[:, :n],
                            lhsT=W[0:64, i, :],
                            rhs=rhs_ap(X, zoff, kind, z, dz, dy, dx, 0, 64),
                            start=(i == 0),
                            stop=(i == n0 - 1),
                            skip_group_check=True,
                            tile_position=(0, 0),
                        )
                        if i < n1:
                            dz, dy, dx = t8_taps[i]
                            nc.tensor.matmul(
                                ps1[:, :n],
                                lhsT=W[64:128, i, :],
                                rhs=rhs_ap(X, zoff, kind, z, dz, dy, dx, 64, 128),
                                start=(i == 0),
                                stop=(i == n1 - 1),
                                skip_group_check=True,
                                tile_position=(64, 0),
                            )
                    tmp = ob_pool.tile([128, 512], F32, name="tmp", tag="tmp")
                    nc.scalar.copy(tmp[:, :n], ps1[:, :n])
                    ob = ob_pool.tile([128, 512], F32, name="ob", tag="ob")
                    nc.vector.tensor_add(ob[:, :n], ps0[:, :n], tmp[:, :n])
                    if kind == "plane":
                        nc.sync.dma_start(out=y_out[:, z, 0:21, :], in_=ob[:, :n])
                    else:
                        # one DMA per z-plane: keeps each transfer one
                        # contiguous run per partition (descriptor-lean)
                        for j in range(6):
                            nc.sync.dma_start(
                                out=y_out[:, zoff + j, 21:24, :],
                                in_=ob[:, j * 72 : (j + 1) * 72],
                            )

            if loop_n is not None:
                with tc.For_i(0, loop_n, 1) as _i:
                    body(_i)
            else:
                body()

    nc.finalize()
    return nc


def _make_in_maps(x, weight):
    w = np.asarray(weight, np.float32).reshape(COUT, CIN, K, K, K)
    wk = np.zeros((128, N_T0, 128), np.float32)
    for i, (dz, dy, dx) in enumerate(TAPS[:N_T0]):
        wk[0:64, i, :] = w[:, :, dz, dy, dx].T
    for i, (dz, dy, dx) in enumerate(TAPS[N_T0:]):
        wk[64:128, i, :] = w[:, :, dz, dy, dx].T

    in_maps = []
    for c in range(N_CORES):
        b, zh = divmod(c, 2)
        z0 = zh * ZS
        xpad = np.zeros((CIN, PW, PW, PW), np.float32)
        xpad[:, 1:25, 1:25, 1:25] = x[b]
        win = xpad[:, z0 : z0 + NP]  # (64, 14, 26, 26)
        X = np.empty((128, NP, PW, PW), np.float32)
        X[0:64] = win
        X[64:128] = win
        in_maps.append({"x": X, "wk": wk})
    return in_maps


def _gather(results):
    out = np.empty((4, COUT, DHW, DHW, DHW), np.float32)
    for c in range(N_CORES):
        b, zh = divmod(c, 2)
        out[b, :, zh * ZS : (zh + 1) * ZS] = results[c]["y"]
    return out


def kernel(x, weight):
    from concourse.bass_utils import run_bass_kernel_spmd

    x = np.asarray(x, np.float32)
    in_maps = _make_in_maps(x, weight)
    nc = _build_program()
    res = run_bass_kernel_spmd(nc, in_maps, list(range(N_CORES)))
    return _gather(res.results)



# revision 2
# speedup vs baseline: 1.4362x; 1.4362x over previous
"""Trainium2 Bass kernel: 3D 'same' convolution (implicit GEMM), v4.

Problem: x (4, 64, 24, 24, 24) f32, weight (1, 128, 1728) f32
         -> out (4, 128, 24, 24, 24) f32  (SAME conv3d, k=3)

Sharding (8 cores): batch (4) x z-halves (2), no inter-core traffic.

Per-core: 27-tap implicit GEMM, PE row-tiled 64x128 with the halves
splitting the output z-planes (L: z 0-5 on partitions 0-63, H: z 6-11
on 64-127). Tap loop is outside the output tiles so each (tap, half)
weight load serves G matmuls; every matmul keeps its explicit
InstLdweights (walrus's separate-Ldweights path uses fast weight load;
suppressing them forces a slower embedded self-load). bf16 inputs, f32
PSUM accumulate. Tiles: per half 6 z-planes x (y 0:21) N=504 plus one
batched remainder (6 z x y 21:24) N=432; 7 PSUM accumulation groups
per half across two rounds (4+3) of the 8 banks, evacuated by ACT (L)
and DVE (H), DMA'd to a tile-contiguous y layout from the SP ring.

The timing loop runs 8 bodies per hardware For_i iteration with
8-deep input rings, so the all-engine barrier + input DMA latency at
the loop edge amortizes across 8 iterations and input DMAs for bodies
2..8 prefetch under compute.
"""

import sys

if "/opt/trn_rl_repo" not in sys.path:
    sys.path.insert(0, "/opt/trn_rl_repo")

import numpy as np

CIN, COUT, K = 64, 128, 3
DHW = 24  # cubic spatial extent
ZS = 12  # z-planes per shard
ZH = 6  # z-planes per half
NPH = 8  # padded z-planes per half window (ZH + 2 halo)
PW = 26  # padded y/x extent
N_CORES = 8
UNROLL = 4  # bodies per hardware-loop iteration

# tap order: all 27 (dz, dy, dx)
TAPS = [(dz, dy, dx) for dz in range(3) for dy in range(3) for dx in range(3)]

# per-half output tiles: ("plane", z) = (z, y 0:21, x) N=504;
# ("rem", None) = (z 0:6, y 21:24, x) N=432.
HALF_TILES = [("plane", z) for z in range(ZH)] + [("rem", None)]
ROUNDS = [HALF_TILES[0:4], HALF_TILES[4:7]]
NPP = 504  # plane-tile columns
NRR = 432  # rem-tile columns
HY = ZH * NPP + NRR  # 3456 output columns per half


def _tile_n_off(kind, z):
    if kind == "plane":
        return NPP, z * NPP
    return NRR, ZH * NPP


def _build_program(loop_n=None):
    """Build the SPMD Bass program (one NeuronCore's view).

    loop_n: if set, run loop_n total bodies as a hardware For_i loop of
    loop_n // UNROLL iterations (loop_n must be a multiple of UNROLL).
    """
    import concourse.tile as tile
    from concourse import bacc, mybir

    F32 = mybir.dt.float32
    BF16 = mybir.dt.bfloat16

    nc = bacc.Bacc("TRN2")
    x_in = nc.declare_dram_parameter("x", [128, NPH, PW, PW], BF16, isOutput=False)
    wk_in = nc.declare_dram_parameter("wk", [128, 27, 128], BF16, isOutput=False)
    y_out = nc.declare_dram_parameter("y", [128, 2 * HY], F32, isOutput=True)

    nbuf = UNROLL if loop_n is not None else 1

    with tile.TileContext(nc) as tc:
        with (
            tc.tile_pool(name="xw", bufs=nbuf) as xw_pool,
            tc.tile_pool(name="ps", bufs=8, space="PSUM") as ps_pool,
            tc.tile_pool(name="ob", bufs=6) as ob_pool,
        ):

            def rhs_ap(X, lo, hi, kind, z, dz, dy, dx):
                if kind == "plane":
                    return X[lo:hi, z + dz, dy : dy + 21, dx : dx + 24]
                return X[lo:hi, dz : dz + ZH, 21 + dy : 24 + dy, dx : dx + 24]

            def load_inputs():
                W = xw_pool.tile([128, 27, 128], BF16, name="W", tag="W", bufs=nbuf)
                nc.sync.dma_start(out=W[:], in_=wk_in[:])
                X = xw_pool.tile([128, NPH, PW, PW], BF16, name="X", tag="X", bufs=nbuf)
                nc.sync.dma_start(out=X[:], in_=x_in[:])
                return W, X

            def body(W, X):
                for rnd_i, rnd in enumerate(ROUNDS):
                    G = len(rnd)
                    ps = [
                        ps_pool.tile([128, 512], F32, name=f"ps{rnd_i}_{g}", tag="ps")
                        for g in range(2 * G)
                    ]  # even = L, odd = H
                    for ti, (dz, dy, dx) in enumerate(TAPS):
                        first, last = ti == 0, ti == len(TAPS) - 1
                        for g, (kind, z) in enumerate(rnd):
                            n, _ = _tile_n_off(kind, z)
                            nc.tensor.matmul(
                                ps[2 * g][:, :n],
                                lhsT=W[0:64, ti, :],
                                rhs=rhs_ap(X, 0, 64, kind, z, dz, dy, dx),
                                start=first,
                                stop=last,
                                skip_group_check=True,
                                tile_position=(0, 0),
                            )
                            nc.tensor.matmul(
                                ps[2 * g + 1][:, :n],
                                lhsT=W[64:128, ti, :],
                                rhs=rhs_ap(X, 64, 128, kind, z, dz, dy, dx),
                                start=first,
                                stop=last,
                                skip_group_check=True,
                                tile_position=(64, 0),
                            )
                    for g, (kind, z) in enumerate(rnd):
                        n, off = _tile_n_off(kind, z)
                        obL = ob_pool.tile([128, 512], F32, name="obL", tag="obL")
                        nc.scalar.copy(obL[:, :n], ps[2 * g][:, :n])
                        nc.scalar.dma_start(
                            out=y_out[:, off : off + n], in_=obL[:, :n]
                        )
                        obH = ob_pool.tile([128, 512], F32, name="obH", tag="obH")
                        nc.vector.tensor_copy(obH[:, :n], ps[2 * g + 1][:, :n])
                        nc.scalar.dma_start(
                            out=y_out[:, HY + off : HY + off + n], in_=obH[:, :n]
                        )

            if loop_n is not None:
                assert loop_n % UNROLL == 0
                with tc.For_i(0, loop_n // UNROLL, 1) as _i:
                    bufs = [load_inputs() for _ in range(UNROLL)]
                    for W, X in bufs:
                        body(W, X)
            else:
                W, X = load_inputs()
                body(W, X)

    nc.finalize()
    return nc


def _make_in_maps(x, weight):
    import ml_dtypes

    BF = ml_dtypes.bfloat16
    w = np.asarray(weight, np.float32).reshape(COUT, CIN, K, K, K)
    wk = np.zeros((128, 27, 128), BF)
    for i, (dz, dy, dx) in enumerate(TAPS):
        wt = w[:, :, dz, dy, dx].T.astype(BF)  # (cin 64, cout 128)
        wk[0:64, i] = wt
        wk[64:128, i] = wt

    x = np.asarray(x, np.float32)
    in_maps = []
    for c in range(N_CORES):
        b, zh = divmod(c, 2)
        z0 = zh * ZS
        xpad = np.zeros((CIN, PW, PW, PW), BF)
        xpad[:, 1:25, 1:25, 1:25] = x[b].astype(BF)
        X = np.empty((128, NPH, PW, PW), BF)
        X[0:64] = xpad[:, z0 : z0 + NPH]  # L: planes z0 .. z0+8
        X[64:128] = xpad[:, z0 + ZH : z0 + ZH + NPH]  # H: planes z0+6 .. z0+14
        in_maps.append({"x": X, "wk": wk})
    return in_maps


def _unshard_half(yh):
    """(128, 3456) tile-ordered -> (128, 6, 24, 24)."""
    planes = yh[:, : ZH * NPP].reshape(128, ZH, 21, 24)
    rem = yh[:, ZH * NPP :].reshape(128, ZH, 3, 24)
    return np.concatenate([planes, rem], axis=2)


def _gather(results):
    out = np.empty((4, COUT, DHW, DHW, DHW), np.float32)
    for c in range(N_CORES):
        b, zh = divmod(c, 2)
        y = results[c]["y"]
        out[b, :, zh * ZS : zh * ZS + ZH] = _unshard_half(y[:, :HY])
        out[b, :, zh * ZS + ZH : (zh + 1) * ZS] = _unshard_half(y[:, HY:])
    return out


def kernel(x, weight):
    from concourse.bass_utils import run_bass_kernel_spmd

    in_maps = _make_in_maps(x, weight)
    nc = _build_program()
    res = run_bass_kernel_spmd(nc, in_maps, list(range(N_CORES)))
    return _gather(res.results)


# revision 4
# speedup vs baseline: 1.6350x; 1.1385x over previous
"""Trainium2 Bass kernel: 3D 'same' convolution (implicit GEMM), v4.

Problem: x (4, 64, 24, 24, 24) f32, weight (1, 128, 1728) f32
         -> out (4, 128, 24, 24, 24) f32  (SAME conv3d, k=3)

Sharding (8 cores): batch (4) x z-halves (2), no inter-core traffic.

Per-core: 27-tap implicit GEMM, PE row-tiled 64x128 with the halves
splitting the output z-planes (L: z 0-5 on partitions 0-63, H: z 6-11
on 64-127). Tap loop is outside the output tiles so each (tap, half)
weight load serves G matmuls; every matmul keeps its explicit
InstLdweights (walrus's separate-Ldweights path uses fast weight load;
suppressing them forces a slower embedded self-load). bf16 inputs, f32
PSUM accumulate. Tiles: per half 6 z-planes x (y 0:21) N=504 plus one
batched remainder (6 z x y 21:24) N=432; 7 PSUM accumulation groups
per half across two rounds (4+3) of the 8 banks, evacuated by ACT (L)
and DVE (H), DMA'd to a tile-contiguous y layout from the SP ring.

The timing loop runs 8 bodies per hardware For_i iteration with
8-deep input rings, so the all-engine barrier + input DMA latency at
the loop edge amortizes across 8 iterations and input DMAs for bodies
2..8 prefetch under compute.
"""

import sys

if "/opt/trn_rl_repo" not in sys.path:
    sys.path.insert(0, "/opt/trn_rl_repo")

import numpy as np

CIN, COUT, K = 64, 128, 3
DHW = 24  # cubic spatial extent
ZS = 12  # z-planes per shard
ZH = 6  # z-planes per half
NPH = 8  # padded z-planes per half window (ZH + 2 halo)
PW = 26  # padded y/x extent
N_CORES = 8
UNROLL = 4  # bodies per hardware-loop iteration

# tap order: all 27 (dz, dy, dx)
TAPS = [(dz, dy, dx) for dz in range(3) for dy in range(3) for dx in range(3)]

# per-half output tiles: ("plane", z) = (z, y 0:21, x) N=504;
# ("rem", None) = (z 0:6, y 21:24, x) N=432.
HALF_TILES = [("plane", z) for z in range(ZH)] + [("rem", None)]
ROUNDS = [HALF_TILES[0:4], HALF_TILES[4:7]]
NPP = 504  # plane-tile columns
NRR = 432  # rem-tile columns
HY = ZH * NPP + NRR  # 3456 output columns per half


def _tile_n_off(kind, z):
    if kind == "plane":
        return NPP, z * NPP
    return NRR, ZH * NPP


def _build_program(loop_n=None):
    """Build the SPMD Bass program (one NeuronCore's view).

    loop_n: if set, run loop_n total bodies as a hardware For_i loop of
    loop_n // UNROLL iterations (loop_n must be a multiple of UNROLL).
    """
    import concourse.tile as tile
    from concourse import bacc, mybir

    F32 = mybir.dt.float32
    BF16 = mybir.dt.bfloat16

    nc = bacc.Bacc("TRN2")
    x_in = nc.declare_dram_parameter("x", [128, NPH, PW, PW], BF16, isOutput=False)
    wk_in = nc.declare_dram_parameter("wk", [128, 27, 128], BF16, isOutput=False)
    y_out = nc.declare_dram_parameter("y", [128, 2 * HY], F32, isOutput=True)

    nbuf = UNROLL if loop_n is not None else 1

    with tile.TileContext(nc) as tc:
        with (
            tc.tile_pool(name="xw", bufs=nbuf) as xw_pool,
            tc.tile_pool(name="ps", bufs=8, space="PSUM") as ps_pool,
            tc.tile_pool(name="ob", bufs=6) as ob_pool,
        ):

            def rhs_ap(X, lo, hi, kind, z, dz, dy, dx):
                if kind == "plane":
                    return X[lo:hi, z + dz, dy : dy + 21, dx : dx + 24]
                return X[lo:hi, dz : dz + ZH, 21 + dy : 24 + dy, dx : dx + 24]

            def load_inputs():
                W = xw_pool.tile([128, 27, 128], BF16, name="W", tag="W", bufs=nbuf)
                nc.sync.dma_start(out=W[:], in_=wk_in[:])
                X = xw_pool.tile([128, NPH, PW, PW], BF16, name="X", tag="X", bufs=nbuf)
                nc.sync.dma_start(out=X[:], in_=x_in[:])
                return W, X

            def body(W, X):
                for rnd_i, rnd in enumerate(ROUNDS):
                    G = len(rnd)
                    ps = [
                        ps_pool.tile([128, 512], F32, name=f"ps{rnd_i}_{g}", tag="ps")
                        for g in range(2 * G)
                    ]  # even = L, odd = H
                    for ti, (dz, dy, dx) in enumerate(TAPS):
                        first, last = ti == 0, ti == len(TAPS) - 1
                        for g, (kind, z) in enumerate(rnd):
                            n, _ = _tile_n_off(kind, z)
                            nc.tensor.matmul(
                                ps[2 * g][:, :n],
                                lhsT=W[0:64, ti, :],
                                rhs=rhs_ap(X, 0, 64, kind, z, dz, dy, dx),
                                start=first,
                                stop=last,
                                skip_group_check=True,
                                tile_position=(0, 0),
                            )
                            nc.tensor.matmul(
                                ps[2 * g + 1][:, :n],
                                lhsT=W[64:128, ti, :],
                                rhs=rhs_ap(X, 64, 128, kind, z, dz, dy, dx),
                                start=first,
                                stop=last,
                                skip_group_check=True,
                                tile_position=(64, 0),
                            )
                    for g, (kind, z) in enumerate(rnd):
                        n, off = _tile_n_off(kind, z)
                        obL = ob_pool.tile([128, 512], F32, name="obL", tag="obL")
                        nc.scalar.copy(obL[:, :n], ps[2 * g][:, :n])
                        nc.scalar.dma_start(
                            out=y_out[:, off : off + n], in_=obL[:, :n]
                        )
                        obH = ob_pool.tile([128, 512], F32, name="obH", tag="obH")
                        nc.vector.tensor_copy(obH[:, :n], ps[2 * g + 1][:, :n])
                        nc.scalar.dma_start(
                            out=y_out[:, HY + off : HY + off + n], in_=obH[:, :n]
                        )

            if loop_n is not None:
                assert loop_n % UNROLL == 0
                # software pipeline: prologue-load every slot outside the
                # loop, then refill each slot right after its body's last
                # read so the DMA overlaps later bodies' compute and no
                # body waits on input DMA after the loop barrier.
                slots = [load_inputs() for _ in range(UNROLL)]
                with tc.For_i(0, loop_n // UNROLL, 1) as _i:
                    for W, X in slots:
                        body(W, X)
                        nc.sync.dma_start(out=W[:], in_=wk_in[:])
                        nc.sync.dma_start(out=X[:], in_=x_in[:])
            else:
                W, X = load_inputs()
                body(W, X)

    nc.finalize()
    return nc


def _make_in_maps(x, weight):
    import ml_dtypes

    BF = ml_dtypes.bfloat16
    w = np.asarray(weight, np.float32).reshape(COUT, CIN, K, K, K)
    wk = np.zeros((128, 27, 128), BF)
    for i, (dz, dy, dx) in enumerate(TAPS):
        wt = w[:, :, dz, dy, dx].T.astype(BF)  # (cin 64, cout 128)
        wk[0:64, i] = wt
        wk[64:128, i] = wt

    x = np.asarray(x, np.float32)
    in_maps = []
    for c in range(N_CORES):
        b, zh = divmod(c, 2)
        z0 = zh * ZS
        xpad = np.zeros((CIN, PW, PW, PW), BF)
        xpad[:, 1:25, 1:25, 1:25] = x[b].astype(BF)
        X = np.empty((128, NPH, PW, PW), BF)
        X[0:64] = xpad[:, z0 : z0 + NPH]  # L: planes z0 .. z0+8
        X[64:128] = xpad[:, z0 + ZH : z0 + ZH + NPH]  # H: planes z0+6 .. z0+14
        in_maps.append({"x": X, "wk": wk})
    return in_maps


def _unshard_half(yh):
    """(128, 3456) tile-ordered -> (128, 6, 24, 24)."""
    planes = yh[:, : ZH * NPP].reshape(128, ZH, 21, 24)
    rem = yh[:, ZH * NPP :].reshape(128, ZH, 3, 24)
    return np.concatenate([planes, rem], axis=2)


def _gather(results):
    out = np.empty((4, COUT, DHW, DHW, DHW), np.float32)
    for c in range(N_CORES):
        b, zh = divmod(c, 2)
        y = results[c]["y"]
        out[b, :, zh * ZS : zh * ZS + ZH] = _unshard_half(y[:, :HY])
        out[b, :, zh * ZS + ZH : (zh + 1) * ZS] = _unshard_half(y[:, HY:])
    return out


def kernel(x, weight):
    from concourse.bass_utils import run_bass_kernel_spmd

    in_maps = _make_in_maps(x, weight)
    nc = _build_program()
    res = run_bass_kernel_spmd(nc, in_maps, list(range(N_CORES)))
    return _gather(res.results)
